# revision 22
# baseline (speedup 1.0000x reference)
"""Trainium2 Bass kernel for nn_DCTFeatureModel.

Math: the reference pipeline (3D DCT-II over [time-in-bin, H, W], mean over
DCT bins, full-receptive-field Conv3d, bias, LeakyReLU) is linear up to the
LeakyReLU, so everything folds into a single small matmul:

    feat[b,s,o] = LeakyReLU( sum_{c,t,i,j} x[b,s,c,t,i,j] * Weff[s,o,t,i,j]
                             + bias[s,o] )
    Weff[s,o,t,i,j] = (1/8) * sum_{f,p,q} Ct[f,t] Cs[p,i] Cs[q,j] W[s,o,f,p,q]

Weff is tiny and computed on host.

Quantization (rel-err budget 2e-2; this lands ~3.3e-3): x ships at 1
byte/element in "count units" x/s (s = 4/127): DCT bins c0..c3 as int8,
c4..c7 as fp8e4. Error-feedback quantization along the c chain (fp8 slices
first, int8 after) makes the device's c-sum accurate to the final int8
residual (~0.3%). The dequant scale s is folded into the fp16 weights.

Device schedule (v4, rates measured on this HW):
- 13 pieces on the sync HWDGE ring; block sizes s1:13+3, s0:11+5 so the
  late-arriving pieces carry little PE work; big fp8 pieces split in two
  for smooth PE feed. Piece-completion sems lag data by 1-3 us (slowest
  SDMA engine), so every consumer queue is issued in measured-readiness
  order.
- PE contracts the fp8 slices RAW (fp16 lhsT x fp8 rhs matmuls accumulate
  the c-sum in PSUM; HW-verified exact): ~170 matmuls, 56-107 ns each,
  LDWEIGHTS fully hidden.
- DVE: u01 everywhere (int8->fp16 ~1.3 ns/col), u23 late blocks, v0 early
  blocks (fp16 2x). GpSimd: u23 for the two big blocks (~2.5 ns/col).
- ACT Prelu table primed at start; both Prelus at the end back-to-back;
  outs on the sync ring.

Sharding: pure data-parallel over batch, 1024/8 = 128 rows per core.
"""

from contextlib import ExitStack

import ml_dtypes
import numpy as np

import concourse.bacc as bacc
import concourse.tile as tile
from concourse import mybir
from concourse.bass_utils import run_bass_kernel_spmd

# Problem shapes (hardcoded per contract)
B = 1024
NCORES = 8
BS = B // NCORES          # 128 batch rows per core
NSW = 2                   # subwindows
NBINS = 8                 # DCT bins (mean-reduced)
NDCT = 32                 # time points per bin
HW = 8
NF = 64                   # conv output filters per subwindow
K = NDCT * HW * HW        # 2048 contraction elements per (s, c)
P = 128                   # partitions
NCHUNK = K // P           # 16 k-chunks of 128 per subwindow
OUT_F = NSW * NF          # 128 output features
SLOPE = 0.02
QSCALE = 4.0 / 127.0      # int8 quant scale (4-sigma clip)

# blocks in STREAM order: (s, chunk_lo, chunk_hi)
BLOCKS = [(1, 0, 13), (0, 0, 11), (0, 11, 16), (1, 13, 16)]
GWS = [(hi - lo) * P for _, lo, hi in BLOCKS]

INT_COLS = sum(4 * gw for gw in GWS)   # c0..c3
FP8_COLS = sum(4 * gw for gw in GWS)   # c4..c7

F32 = mybir.dt.float32
F16 = mybir.dt.float16
I8 = mybir.dt.int8
FP8 = mybir.dt.float8e4
NP_F16 = np.float16
NP_FP8 = ml_dtypes.float8_e4m3fn

_cached = None
last_results = None


def _dct2(N):
    n = np.arange(N, dtype=np.float64)
    k = np.arange(N, dtype=np.float64)
    return 2.0 * np.cos(np.pi * (2.0 * n[None, :] + 1.0) * k[:, None] / (2.0 * N))


# host-side piece list: (kind, block, name, c-slices)
# int pieces come from xi (c-slices of 0..3), fp8 pieces from xf (0..3 = c4..c7)
PIECES = [
    ("f", 0, "f8a", (0, 1)),
    ("i", 0, "int", (0, 1, 2, 3)),
    ("f", 0, "f8b", (2, 3)),
    ("f", 1, "f8a", (0, 1)),
    ("i", 1, "int", (0, 1, 2, 3)),
    ("f", 1, "f8b", (2, 3)),
    ("i", 2, "int", (0, 1, 2, 3)),
    ("f", 2, "f8", (0, 1, 2, 3)),
    ("i", 3, "int", (0, 1, 2, 3)),
    ("f", 3, "f8", (0, 1, 2, 3)),
]

# per-block column base in the block-major w layout
W_BASE = [0]
for _s, _lo, _hi in BLOCKS:
    W_BASE.append(W_BASE[-1] + (_hi - _lo) * NF)
W01_COLS = W_BASE[2]              # blocks 0+1
W23_COLS = W_BASE[4] - W_BASE[2]  # blocks 2+3


def _kernel_body(tc, xi, xf, w, bias, out):
    nc = tc.nc
    with ExitStack() as ctx:
        const_pool = ctx.enter_context(tc.tile_pool(name="const", bufs=1))
        xpool = ctx.enter_context(tc.tile_pool(name="xp", bufs=1))
        upool = ctx.enter_context(tc.tile_pool(name="up", bufs=1))
        pft_pool = ctx.enter_context(tc.tile_pool(name="pft", bufs=1, space="PSUM"))

        # consts ride the scalar (ACT) HWDGE ring - off the x stream ring.
        # w comes in 2 block-major slices: one 512 KB lump gets round-robin
        # interleaved into the x stream at a different point per SDMA engine
        # (~2 us of piece-completion skew), while 4+ small DMAs overflow the
        # ~6-deep queue and stall the descriptor generator.
        w01_sb = const_pool.tile([P, W01_COLS], F16, tag="w01", name="w01")
        nc.scalar.dma_start(out=w01_sb, in_=w[:, 0:W01_COLS])
        w23_sb = const_pool.tile([P, W23_COLS], F16, tag="w23", name="w23")
        nc.scalar.dma_start(out=w23_sb, in_=w[:, W01_COLS:W01_COLS + W23_COLS])
        w_sb = []
        for bi in range(4):
            src = w01_sb if bi < 2 else w23_sb
            base = W_BASE[bi] - (0 if bi < 2 else W01_COLS)
            w_sb.append((src, base))
        bias_sb = const_pool.tile([OUT_F, 1], F32)
        nc.scalar.dma_start(out=bias_sb, in_=bias)
        alpha_sb = const_pool.tile([OUT_F, 1], F32)
        nc.gpsimd.memset(alpha_sb, SLOPE)
        # prime the ACT Prelu table so ACT_TABLE_LOAD (~1.3us) runs early
        prime_sb = const_pool.tile([OUT_F, 1], F32)
        nc.scalar.activation(
            prime_sb,
            alpha_sb,
            mybir.ActivationFunctionType.Prelu,
            bias=alpha_sb,
            alpha=alpha_sb,
        )

        out_sb = const_pool.tile([OUT_F, BS], F32)
        psum_feat = [
            pft_pool.tile([NF, BS], F32, tag=f"feat{s}", name=f"psum_feat{s}")
            for s in range(NSW)
        ]

        # pre-warm the PE: the HAM clock gate needs ~3.4us of sustained
        # activity to ramp 1.2 -> 2.4 GHz; burn dummy matmuls on a memset
        # tile so the real matmuls start warm (56 vs 107 ns each).
        warm_sb = const_pool.tile([P, NF], F16, tag="warm", name="warm")
        nc.gpsimd.memset(warm_sb, 0.0)
        psum_warm = pft_pool.tile([NF, NF], F32, tag="warm", name="psum_warm")
        NWARM = 32
        for i in range(NWARM):
            nc.tensor.matmul(
                psum_warm, lhsT=warm_sb, rhs=warm_sb,
                start=(i == 0), stop=(i == NWARM - 1),
            )
        # prime the DVE as well (first DVE op pays ~0.3us of ucode warmup)
        warm_v = const_pool.tile([P, NF], F16, tag="warmv", name="warmv")
        nc.vector.tensor_add(out=warm_v, in0=warm_sb, in1=warm_sb)

        # stream pieces, strict FIFO on the sync ring
        tiles = {}
        ioff = foff = 0
        for kind, bi, nm, cs in PIECES:
            gw = GWS[bi]
            ncols = len(cs) * gw
            if kind == "i":
                t = xpool.tile([P, ncols], I8, tag=f"x{bi}{nm}", name=f"x{bi}{nm}")
                nc.sync.dma_start(out=t, in_=xi[:, ioff:ioff + ncols])
                ioff += ncols
            else:
                t = xpool.tile([P, ncols], FP8, tag=f"x{bi}{nm}", name=f"x{bi}{nm}")
                nc.sync.dma_start(out=t, in_=xf[:, foff:foff + ncols])
                foff += ncols
            tiles[(bi, nm)] = t

        # --- adds (issue order = measured readiness; queues are in-order) --
        def pair_add(eng, nm, bi, src, off):
            gw = GWS[bi]
            t = upool.tile([P, gw], F16, tag=f"{nm}_{bi}", name=f"{nm}_{bi}")
            eng.tensor_add(
                out=t, in0=src[:, off * gw:(off + 1) * gw],
                in1=src[:, (off + 1) * gw:(off + 2) * gw],
            )
            return t

        # gpsimd: u23 for the big s1 block only (Q7 is ~2x slower than DVE)
        u23_0 = pair_add(nc.gpsimd, "u23", 0, tiles[(0, "int")], 2)
        # DVE (issue order = measured readiness)
        u01_0 = pair_add(nc.vector, "u01", 0, tiles[(0, "int")], 0)
        v0_0 = upool.tile([P, GWS[0]], F16, tag="v0_0", name="v0_0")
        nc.vector.tensor_add(out=v0_0, in0=u01_0, in1=u23_0)
        u01_1 = pair_add(nc.vector, "u01", 1, tiles[(1, "int")], 0)
        u23_1 = pair_add(nc.vector, "u23", 1, tiles[(1, "int")], 2)
        u01_2 = pair_add(nc.vector, "u01", 2, tiles[(2, "int")], 0)
        u23_2 = pair_add(nc.vector, "u23", 2, tiles[(2, "int")], 2)
        u01_3 = pair_add(nc.vector, "u01", 3, tiles[(3, "int")], 0)
        u23_3 = pair_add(nc.vector, "u23", 3, tiles[(3, "int")], 2)

        # --- matmuls ------------------------------------------------------
        n_mm = {0: 0, 1: 0}
        for bi, (s, lo, hi) in enumerate(BLOCKS):
            nch = hi - lo
            n_mm[s] += 4 * nch + (nch if bi == 0 else 2 * nch)
        mm_seen = {0: 0, 1: 0}

        def mm(bi, chin, rhs):
            s = BLOCKS[bi][0]
            wt, base = w_sb[bi]
            mm_seen[s] += 1
            nc.tensor.matmul(
                psum_feat[s],
                lhsT=wt[:, base + chin * NF:base + (chin + 1) * NF],
                rhs=rhs,
                start=(mm_seen[s] == 1),
                stop=(mm_seen[s] == n_mm[s]),
            )

        def fp8_group(bi, nm, njs):
            _, lo, hi = BLOCKS[bi]
            gw = GWS[bi]
            f8 = tiles[(bi, nm)]
            for j in range(njs):
                for chin in range(hi - lo):
                    mm(bi, chin, f8[:, j * gw + chin * P:j * gw + (chin + 1) * P])

        def red_group(bi, src):
            _, lo, hi = BLOCKS[bi]
            for chin in range(hi - lo):
                mm(bi, chin, src[:, chin * P:(chin + 1) * P])

        # PE queue in expected-readiness order
        fp8_group(0, "f8a", 2)
        fp8_group(0, "f8b", 2)
        fp8_group(1, "f8a", 2)
        red_group(0, v0_0)
        red_group(1, u01_1)
        fp8_group(1, "f8b", 2)
        red_group(1, u23_1)
        fp8_group(2, "f8", 4)
        red_group(2, u01_2)
        red_group(2, u23_2)                # s0 stop
        red_group(3, u01_3)
        red_group(3, u23_3)
        fp8_group(3, "f8", 4)              # s1 stop (last piece's own work)

        assert mm_seen[0] == n_mm[0] and mm_seen[1] == n_mm[1]

        # --- epilogue: Prelus back-to-back on ACT, ONE out DMA on sync ----
        for s in range(NSW):
            nc.scalar.activation(
                out_sb[s * NF:(s + 1) * NF, :],
                psum_feat[s],
                mybir.ActivationFunctionType.Prelu,
                bias=bias_sb[s * NF:(s + 1) * NF, :],
                alpha=alpha_sb[s * NF:(s + 1) * NF, :],
            )
        nc.sync.dma_start(out=out, in_=out_sb)


def _build():
    global _cached
    if _cached is not None:
        return _cached
    nc = bacc.Bacc(
        "TRN2",
        target_bir_lowering=False,
        debug=False,
        enable_asserts=False,
        num_devices=NCORES,
    )
    xi_ap = nc.dram_tensor("xi", [P, INT_COLS], I8, kind="ExternalInput").ap()
    xf_ap = nc.dram_tensor("xf", [P, FP8_COLS], FP8, kind="ExternalInput").ap()
    w_ap = nc.dram_tensor("w", [P, NSW * NCHUNK * NF], F16, kind="ExternalInput").ap()
    b_ap = nc.dram_tensor("bias", [OUT_F, 1], F32, kind="ExternalInput").ap()
    out_ap = nc.dram_tensor("out", [OUT_F, BS], F32, kind="ExternalOutput").ap()
    with tile.TileContext(nc, trace_sim=False) as tc:
        _kernel_body(tc, xi_ap, xf_ap, w_ap, b_ap, out_ap)
    nc.compile()
    _cached = nc
    return nc


def kernel(x, W, b):
    global last_results
    assert x.shape == (B, 1, NSW * NBINS * NDCT, HW, HW), x.shape
    nc = _build()

    # Host-side folding of the DCT matrices into the conv weights (tiny).
    Ct = _dct2(NDCT)                       # [f, t]
    Cs = _dct2(HW)                         # [p, i]
    Weff = np.einsum(
        "ft,pi,qj,sofpq->sotij", Ct, Cs, Cs, W.astype(np.float64), optimize=True
    ) / float(NBINS)
    Weff_k = Weff.reshape(NSW, NF, K) * QSCALE   # fold dequant scale
    # block-major lhsT layout: w[p, W_BASE[bi] + chin*NF + o]
    #   = Weff_k[s, o, (lo+chin)*128 + p]
    wc = Weff_k.reshape(NSW, NF, NCHUNK, P)
    blocks_w = []
    for s, lo, hi in BLOCKS:
        blocks_w.append(wc[s, :, lo:hi, :].transpose(2, 1, 0).reshape(P, (hi - lo) * NF))
    w_dev = np.ascontiguousarray(np.concatenate(blocks_w, axis=1)).astype(NP_F16)
    bias_dev = np.ascontiguousarray(b.reshape(OUT_F, 1)).astype(np.float32)

    # Error-feedback quantization along c (fp8 slices first so their larger
    # residuals are absorbed by the later int8 slices).
    xs = x.reshape(B, NSW, NBINS, K).astype(np.float32) / np.float32(QSCALE)
    qi = np.zeros((B, NSW, 4, K), dtype=np.int8)
    qf = np.zeros((B, NSW, 4, K), dtype=NP_FP8)
    e = np.zeros((B, NSW, K), dtype=np.float32)
    for j, c in enumerate((4, 5, 6, 7)):
        v = xs[:, :, c] + e
        qc = v.astype(NP_FP8)
        e = v - qc.astype(np.float32)
        qf[:, :, j] = qc
    for c in range(4):
        v = xs[:, :, c] + e
        qc = np.clip(np.round(v), -127, 127)
        e = v - qc
        qi[:, :, c] = qc.astype(np.int8)

    qi = qi.reshape(B, NSW, 4, NCHUNK, P)
    qf = qf.reshape(B, NSW, 4, NCHUNK, P)

    in_maps = []
    for i in range(NCORES):
        sl = slice(i * BS, (i + 1) * BS)
        icols, fcols = [], []
        for kind, bi, nm, cs in PIECES:
            s, lo, hi = BLOCKS[bi]
            src = qi if kind == "i" else qf
            t = src[sl, s][:, list(cs), lo:hi, :]        # [b, nc, ch, kin]
            t = t.transpose(3, 1, 2, 0).reshape(P, len(cs) * (hi - lo) * BS)
            (icols if kind == "i" else fcols).append(t)
        xi_dev = np.ascontiguousarray(np.concatenate(icols, axis=1))
        xf_dev = np.ascontiguousarray(np.concatenate(fcols, axis=1))
        in_maps.append({"xi": xi_dev, "xf": xf_dev, "w": w_dev, "bias": bias_dev})
    res = run_bass_kernel_spmd(nc, in_maps, core_ids=list(range(NCORES)))
    last_results = res
    # device emits [s*64+o, b] per core; transpose back to [b, s*64+o]
    return np.concatenate([r["out"].T for r in res.results], axis=0)


# revision 28
# speedup vs baseline: 1.1620x; 1.1620x over previous
"""Trainium2 Bass kernel for nn_DCTFeatureModel.

Math: the reference pipeline (3D DCT-II over [time-in-bin, H, W], mean over
DCT bins, full-receptive-field Conv3d, bias, LeakyReLU) is linear up to the
LeakyReLU, so everything folds into a single small matmul:

    feat[b,s,o] = LeakyReLU( sum_{c,t,i,j} x[b,s,c,t,i,j] * Weff[s,o,t,i,j]
                             + bias[s,o] )
    Weff[s,o,t,i,j] = (1/8) * sum_{f,p,q} Ct[f,t] Cs[p,i] Cs[q,j] W[s,o,f,p,q]

Weff is tiny and computed on host.

Quantization (rel-err budget 2e-2; this lands ~3.3e-3): x ships at 1
byte/element in "count units" x/s (s = 4/127): DCT bins c0..c3 as int8,
c4..c7 as fp8e4. Error-feedback quantization along the c chain (fp8 slices
first, int8 after) makes the device's c-sum accurate to the final int8
residual (~0.3%). The dequant scale s is folded into the fp16 weights.

Device schedule (final, rates measured on this HW; ~33 us vs 46.7 us
bf16 baseline, now chip-level-HBM-bound: 8 cores pull ~2.3 TB/s):
- 12 x-pieces on the sync HWDGE ring, each big block led by half its fp8
  so the PE starts ~4 us earlier; block sizes s1:13+3, s0:11+5 so the
  late-arriving pieces carry little PE work. Piece-completion sems lag
  the data by the slowest SDMA engine, so every consumer queue is issued
  in measured-readiness order (all queues are in-order).
- w rides the scalar HWDGE ring in 2 block-major slices: one 512 KB lump
  gets round-robin interleaved into the x stream at a different point per
  SDMA engine (~2 us of sem skew), while 4+ small DMAs overflow the
  ~6-deep shared queue and stall the descriptor generator.
- PE contracts the fp8 slices RAW (fp16 lhsT x fp8 rhs matmuls accumulate
  the c-sum in PSUM; HW-verified exact): 179 matmuls, 56-107 ns each,
  LDWEIGHTS fully hidden; 32 dummy matmuls at kernel start ramp the HAM
  clock gate 1.2 -> 2.4 GHz before real work arrives.
- DVE: u01=c0+c1 everywhere (int8->fp16 ~1.3 ns/col), u23 for the three
  later blocks, v0=u01+u23 for block 0 only (fp16 2x). GpSimd: u23 of the
  big s1 block (~2.5 ns/col). Unreduced u01/u23 go straight to the PE.
- ACT Prelu table primed at start (else its 1.3 us load lands in the
  tail); both Prelus back-to-back at the end; one merged out DMA.

Sharding: pure data-parallel over batch, 1024/8 = 128 rows per core.
"""

from contextlib import ExitStack

import ml_dtypes
import numpy as np

import concourse.bacc as bacc
import concourse.tile as tile
from concourse import mybir
from concourse.bass_utils import run_bass_kernel_spmd

# Problem shapes (hardcoded per contract)
B = 1024
NCORES = 8
BS = B // NCORES          # 128 batch rows per core
NSW = 2                   # subwindows
NBINS = 8                 # DCT bins (mean-reduced)
NDCT = 32                 # time points per bin
HW = 8
NF = 64                   # conv output filters per subwindow
K = NDCT * HW * HW        # 2048 contraction elements per (s, c)
P = 128                   # partitions
NCHUNK = K // P           # 16 k-chunks of 128 per subwindow
OUT_F = NSW * NF          # 128 output features
SLOPE = 0.02
QSCALE = 4.0 / 127.0      # int8 quant scale (4-sigma clip)

# blocks in STREAM order: (s, chunk_lo, chunk_hi)
BLOCKS = [(1, 0, 13), (0, 0, 11), (0, 11, 16), (1, 13, 16)]
GWS = [(hi - lo) * P for _, lo, hi in BLOCKS]

INT_COLS = sum(4 * gw for gw in GWS)   # c0..c3
FP8_COLS = sum(4 * gw for gw in GWS)   # c4..c7

F32 = mybir.dt.float32
F16 = mybir.dt.float16
I8 = mybir.dt.int8
FP8 = mybir.dt.float8e4
NP_F16 = np.float16
NP_FP8 = ml_dtypes.float8_e4m3fn

_cached = None
last_results = None


def _dct2(N):
    n = np.arange(N, dtype=np.float64)
    k = np.arange(N, dtype=np.float64)
    return 2.0 * np.cos(np.pi * (2.0 * n[None, :] + 1.0) * k[:, None] / (2.0 * N))


# host-side piece list: (kind, block, name, c-slices)
# int pieces come from xi (c-slices of 0..3), fp8 pieces from xf (0..3 = c4..c7)
PIECES = [
    ("f", 0, "f8a", (0, 1)),
    ("i", 0, "c01", (0, 1)),
    ("i", 0, "c23", (2, 3)),
    ("f", 0, "f8b", (2, 3)),
    ("f", 1, "f8a", (0, 1)),
    ("i", 1, "c01", (0, 1)),
    ("i", 1, "c23", (2, 3)),
    ("f", 1, "f8b", (2, 3)),
    ("i", 2, "int", (0, 1, 2, 3)),
    ("f", 2, "f8", (0, 1, 2, 3)),
    ("i", 3, "int", (0, 1, 2, 3)),
    ("f", 3, "f8", (0, 1, 2, 3)),
]

# per-block column base in the block-major w layout
W_BASE = [0]
for _s, _lo, _hi in BLOCKS:
    W_BASE.append(W_BASE[-1] + (_hi - _lo) * NF)
W01_COLS = W_BASE[2]              # blocks 0+1
W23_COLS = W_BASE[4] - W_BASE[2]  # blocks 2+3


def _kernel_body(tc, xpieces, w01, w23, bias, out):
    nc = tc.nc
    with ExitStack() as ctx:
        const_pool = ctx.enter_context(tc.tile_pool(name="const", bufs=1))
        xpool = ctx.enter_context(tc.tile_pool(name="xp", bufs=1))
        upool = ctx.enter_context(tc.tile_pool(name="up", bufs=1))
        pft_pool = ctx.enter_context(tc.tile_pool(name="pft", bufs=1, space="PSUM"))

        # consts ride the scalar (ACT) HWDGE ring - off the x stream ring.
        # w comes in 2 block-major slices: one 512 KB lump gets round-robin
        # interleaved into the x stream at a different point per SDMA engine
        # (~2 us of piece-completion skew), while 4+ small DMAs overflow the
        # ~6-deep queue and stall the descriptor generator.
        w01_sb = const_pool.tile([P, W01_COLS], F16, tag="w01", name="w01")
        nc.scalar.dma_start(out=w01_sb, in_=w01)
        w23_sb = const_pool.tile([P, W23_COLS], F16, tag="w23", name="w23")
        nc.scalar.dma_start(out=w23_sb, in_=w23)
        w_sb = []
        for bi in range(4):
            src = w01_sb if bi < 2 else w23_sb
            base = W_BASE[bi] - (0 if bi < 2 else W01_COLS)
            w_sb.append((src, base))
        bias_sb = const_pool.tile([OUT_F, 1], F32)
        nc.scalar.dma_start(out=bias_sb, in_=bias)
        alpha_sb = const_pool.tile([OUT_F, 1], F32)
        nc.gpsimd.memset(alpha_sb, SLOPE)
        # prime the ACT Prelu table so ACT_TABLE_LOAD (~1.3us) runs early
        prime_sb = const_pool.tile([OUT_F, 1], F32)
        nc.scalar.activation(
            prime_sb,
            alpha_sb,
            mybir.ActivationFunctionType.Prelu,
            bias=alpha_sb,
            alpha=alpha_sb,
        )

        out_sb = const_pool.tile([OUT_F, BS], F32)
        psum_feat = [
            pft_pool.tile([NF, BS], F32, tag=f"feat{s}", name=f"psum_feat{s}")
            for s in range(NSW)
        ]

        # pre-warm the PE: the HAM clock gate needs ~3.4us of sustained
        # activity to ramp 1.2 -> 2.4 GHz; burn dummy matmuls on a memset
        # tile so the real matmuls start warm (56 vs 107 ns each).
        warm_sb = const_pool.tile([P, NF], F16, tag="warm", name="warm")
        nc.gpsimd.memset(warm_sb, 0.0)
        psum_warm = pft_pool.tile([NF, NF], F32, tag="warm", name="psum_warm")
        NWARM = 32
        for i in range(NWARM):
            nc.tensor.matmul(
                psum_warm, lhsT=warm_sb, rhs=warm_sb,
                start=(i == 0), stop=(i == NWARM - 1),
            )
        # prime the DVE as well (first DVE op pays ~0.3us of ucode warmup)
        warm_v = const_pool.tile([P, NF], F16, tag="warmv", name="warmv")
        nc.vector.tensor_add(out=warm_v, in0=warm_sb, in1=warm_sb)

        # stream pieces, strict FIFO on the sync ring; each piece is its own
        # CONTIGUOUS dram tensor (a column-slice of one big [P, COLS] tensor
        # makes the HBM side read 2-6 KB segments with 16 KB row strides)
        tiles = {}
        for pi, (kind, bi, nm, cs) in enumerate(PIECES):
            gw = GWS[bi]
            ncols = len(cs) * gw
            dt = I8 if kind == "i" else FP8
            t = xpool.tile([P, ncols], dt, tag=f"x{bi}{nm}", name=f"x{bi}{nm}")
            nc.sync.dma_start(out=t, in_=xpieces[pi])
            tiles[(bi, nm)] = t

        # --- adds (issue order = measured readiness; queues are in-order) --
        def pair_add(eng, nm, bi, src, off):
            gw = GWS[bi]
            t = upool.tile([P, gw], F16, tag=f"{nm}_{bi}", name=f"{nm}_{bi}")
            eng.tensor_add(
                out=t, in0=src[:, off * gw:(off + 1) * gw],
                in1=src[:, (off + 1) * gw:(off + 2) * gw],
            )
            return t

        # gpsimd: u23 for the big s1 block only (Q7 is ~2x slower than DVE)
        u23_0 = pair_add(nc.gpsimd, "u23", 0, tiles[(0, "c23")], 0)
        # DVE (issue order = measured readiness)
        u01_0 = pair_add(nc.vector, "u01", 0, tiles[(0, "c01")], 0)
        v0_0 = upool.tile([P, GWS[0]], F16, tag="v0_0", name="v0_0")
        nc.vector.tensor_add(out=v0_0, in0=u01_0, in1=u23_0)
        u01_1 = pair_add(nc.vector, "u01", 1, tiles[(1, "c01")], 0)
        u23_1 = pair_add(nc.vector, "u23", 1, tiles[(1, "c23")], 0)
        u01_2 = pair_add(nc.vector, "u01", 2, tiles[(2, "int")], 0)
        u23_2 = pair_add(nc.vector, "u23", 2, tiles[(2, "int")], 2)
        u01_3 = pair_add(nc.vector, "u01", 3, tiles[(3, "int")], 0)
        u23_3 = pair_add(nc.vector, "u23", 3, tiles[(3, "int")], 2)

        # --- matmuls ------------------------------------------------------
        n_mm = {0: 0, 1: 0}
        for bi, (s, lo, hi) in enumerate(BLOCKS):
            nch = hi - lo
            n_mm[s] += 4 * nch + (nch if bi == 0 else 2 * nch)
        mm_seen = {0: 0, 1: 0}

        def mm(bi, chin, rhs):
            s = BLOCKS[bi][0]
            wt, base = w_sb[bi]
            mm_seen[s] += 1
            nc.tensor.matmul(
                psum_feat[s],
                lhsT=wt[:, base + chin * NF:base + (chin + 1) * NF],
                rhs=rhs,
                start=(mm_seen[s] == 1),
                stop=(mm_seen[s] == n_mm[s]),
            )

        def fp8_group(bi, nm, njs):
            _, lo, hi = BLOCKS[bi]
            gw = GWS[bi]
            f8 = tiles[(bi, nm)]
            for j in range(njs):
                for chin in range(hi - lo):
                    mm(bi, chin, f8[:, j * gw + chin * P:j * gw + (chin + 1) * P])

        def red_group(bi, src):
            _, lo, hi = BLOCKS[bi]
            for chin in range(hi - lo):
                mm(bi, chin, src[:, chin * P:(chin + 1) * P])

        # PE queue in expected-readiness order
        fp8_group(0, "f8a", 2)
        fp8_group(0, "f8b", 2)
        fp8_group(1, "f8a", 2)
        red_group(0, v0_0)
        red_group(1, u01_1)
        fp8_group(1, "f8b", 2)
        red_group(1, u23_1)
        fp8_group(2, "f8", 4)
        red_group(2, u01_2)
        red_group(2, u23_2)                # s0 stop
        red_group(3, u01_3)
        red_group(3, u23_3)
        fp8_group(3, "f8", 4)              # s1 stop (last piece's own work)

        assert mm_seen[0] == n_mm[0] and mm_seen[1] == n_mm[1]

        # --- epilogue: Prelus back-to-back on ACT, ONE out DMA on sync ----
        for s in range(NSW):
            nc.scalar.activation(
                out_sb[s * NF:(s + 1) * NF, :],
                psum_feat[s],
                mybir.ActivationFunctionType.Prelu,
                bias=bias_sb[s * NF:(s + 1) * NF, :],
                alpha=alpha_sb[s * NF:(s + 1) * NF, :],
            )
        nc.sync.dma_start(out=out, in_=out_sb)


def _build():
    global _cached
    if _cached is not None:
        return _cached
    nc = bacc.Bacc(
        "TRN2",
        target_bir_lowering=False,
        debug=False,
        enable_asserts=False,
        num_devices=NCORES,
    )
    xp_aps = []
    for pi, (kind, bi, nm, cs) in enumerate(PIECES):
        ncols = len(cs) * GWS[bi]
        dt = I8 if kind == "i" else FP8
        xp_aps.append(
            nc.dram_tensor(f"xp{pi}", [P, ncols], dt, kind="ExternalInput").ap()
        )
    w01_ap = nc.dram_tensor("w01", [P, W01_COLS], F16, kind="ExternalInput").ap()
    w23_ap = nc.dram_tensor("w23", [P, W23_COLS], F16, kind="ExternalInput").ap()
    b_ap = nc.dram_tensor("bias", [OUT_F, 1], F32, kind="ExternalInput").ap()
    out_ap = nc.dram_tensor("out", [OUT_F, BS], F32, kind="ExternalOutput").ap()
    with tile.TileContext(nc, trace_sim=False) as tc:
        _kernel_body(tc, xp_aps, w01_ap, w23_ap, b_ap, out_ap)
    nc.compile()
    _cached = nc
    return nc


def kernel(x, W, b):
    global last_results
    assert x.shape == (B, 1, NSW * NBINS * NDCT, HW, HW), x.shape
    nc = _build()

    # Host-side folding of the DCT matrices into the conv weights (tiny).
    Ct = _dct2(NDCT)                       # [f, t]
    Cs = _dct2(HW)                         # [p, i]
    Weff = np.einsum(
        "ft,pi,qj,sofpq->sotij", Ct, Cs, Cs, W.astype(np.float64), optimize=True
    ) / float(NBINS)
    Weff_k = Weff.reshape(NSW, NF, K) * QSCALE   # fold dequant scale
    # block-major lhsT layout: w[p, W_BASE[bi] + chin*NF + o]
    #   = Weff_k[s, o, (lo+chin)*128 + p]
    wc = Weff_k.reshape(NSW, NF, NCHUNK, P)
    blocks_w = []
    for s, lo, hi in BLOCKS:
        blocks_w.append(wc[s, :, lo:hi, :].transpose(2, 1, 0).reshape(P, (hi - lo) * NF))
    w_dev = np.ascontiguousarray(np.concatenate(blocks_w, axis=1)).astype(NP_F16)
    bias_dev = np.ascontiguousarray(b.reshape(OUT_F, 1)).astype(np.float32)

    # Error-feedback quantization along c (fp8 slices first so their larger
    # residuals are absorbed by the later int8 slices).
    xs = x.reshape(B, NSW, NBINS, K).astype(np.float32) / np.float32(QSCALE)
    qi = np.zeros((B, NSW, 4, K), dtype=np.int8)
    qf = np.zeros((B, NSW, 4, K), dtype=NP_FP8)
    e = np.zeros((B, NSW, K), dtype=np.float32)
    for j, c in enumerate((4, 5, 6, 7)):
        v = xs[:, :, c] + e
        qc = v.astype(NP_FP8)
        e = v - qc.astype(np.float32)
        qf[:, :, j] = qc
    for c in range(4):
        v = xs[:, :, c] + e
        qc = np.clip(np.round(v), -127, 127)
        e = v - qc
        qi[:, :, c] = qc.astype(np.int8)

    qi = qi.reshape(B, NSW, 4, NCHUNK, P)
    qf = qf.reshape(B, NSW, 4, NCHUNK, P)

    w01_dev = np.ascontiguousarray(w_dev[:, 0:W01_COLS])
    w23_dev = np.ascontiguousarray(w_dev[:, W01_COLS:W01_COLS + W23_COLS])
    in_maps = []
    for i in range(NCORES):
        sl = slice(i * BS, (i + 1) * BS)
        m = {"w01": w01_dev, "w23": w23_dev, "bias": bias_dev}
        for pi, (kind, bi, nm, cs) in enumerate(PIECES):
            s, lo, hi = BLOCKS[bi]
            src = qi if kind == "i" else qf
            t = src[sl, s][:, list(cs), lo:hi, :]        # [b, nc, ch, kin]
            t = t.transpose(3, 1, 2, 0).reshape(P, len(cs) * (hi - lo) * BS)
            m[f"xp{pi}"] = np.ascontiguousarray(t)
        in_maps.append(m)
    res = run_bass_kernel_spmd(nc, in_maps, core_ids=list(range(NCORES)))
    last_results = res
    # device emits [s*64+o, b] per core; transpose back to [b, s*64+o]
    return np.concatenate([r["out"].T for r in res.results], axis=0)


# revision 32
# speedup vs baseline: 1.1767x; 1.0127x over previous
"""Trainium2 Bass kernel for nn_DCTFeatureModel.

Math: the reference pipeline (3D DCT-II over [time-in-bin, H, W], mean over
DCT bins, full-receptive-field Conv3d, bias, LeakyReLU) is linear up to the
LeakyReLU, so everything folds into a single small matmul:

    feat[b,s,o] = LeakyReLU( sum_{c,t,i,j} x[b,s,c,t,i,j] * Weff[s,o,t,i,j]
                             + bias[s,o] )
    Weff[s,o,t,i,j] = (1/8) * sum_{f,p,q} Ct[f,t] Cs[p,i] Cs[q,j] W[s,o,f,p,q]

Weff is tiny and computed on host.

Quantization (rel-err budget 2e-2; this lands ~3.3e-3): x ships at 1
byte/element in "count units" x/s (s = 4/127): DCT bins c0..c3 as int8,
c4..c7 as fp8e4. Error-feedback quantization along the c chain (fp8 slices
first, int8 after) makes the device's c-sum accurate to the final int8
residual (~0.3%). The dequant scale s is folded into the fp16 weights.

Device schedule (final, ~33 us vs 46.7 us bf16 baseline; now chip-level
HBM-bound - 8 cores pull ~2.3 TB/s aggregate):
- 12 x-pieces on the sync HWDGE ring, each big block led by half its fp8
  so the PE starts early; block sizes s1:13+3, s0:11+5 so late-arriving
  pieces carry little PE work. Piece-completion sems lag the data by the
  slowest SDMA engine, so every consumer queue is issued in
  measured-readiness order (all queues are in-order).
- w rides the scalar HWDGE ring in 2 block-major slices: one 512 KB lump
  gets round-robin interleaved into the x stream at a different point per
  SDMA engine (~2 us sem skew); 4+ small DMAs overflow the ~6-deep queue.
- PE contracts the fp8 slices RAW (fp16 lhsT x fp8 rhs matmuls accumulate
  the c-sum in PSUM; HW-verified exact): 179 matmuls, 56-107 ns each,
  LDWEIGHTS fully hidden; 32 dummy matmuls at start ramp the HAM clock
  gate 1.2 -> 2.4 GHz before real work arrives.
- DVE: u01=c0+c1 everywhere (int8->fp16 ~1.3 ns/col), u23 for the three
  later blocks, v0=u01+u23 for block 0 (fp16 2x). GpSimd: u23 of the big
  s1 block (~2.5 ns/col). Unreduced u01/u23 go straight to the PE.
- ACT Prelu table primed at start (else its 1.3 us load lands in the
  tail); both Prelus back-to-back at the end; one merged out DMA.

Sharding: pure data-parallel over batch, 1024/8 = 128 rows per core.
"""

from contextlib import ExitStack

import ml_dtypes
import numpy as np

import concourse.bacc as bacc
import concourse.tile as tile
from concourse import mybir
from concourse.bass_utils import run_bass_kernel_spmd

# Problem shapes (hardcoded per contract)
B = 1024
NCORES = 8
BS = B // NCORES          # 128 batch rows per core
NSW = 2                   # subwindows
NBINS = 8                 # DCT bins (mean-reduced)
NDCT = 32                 # time points per bin
HW = 8
NF = 64                   # conv output filters per subwindow
K = NDCT * HW * HW        # 2048 contraction elements per (s, c)
P = 128                   # partitions
NCHUNK = K // P           # 16 k-chunks of 128 per subwindow
OUT_F = NSW * NF          # 128 output features
SLOPE = 0.02
QSCALE = 4.0 / 127.0      # int8 quant scale (4-sigma clip)

# blocks in STREAM order: (s, chunk_lo, chunk_hi)
BLOCKS = [(1, 0, 13), (0, 0, 11), (0, 11, 16), (1, 13, 16)]
GWS = [(hi - lo) * P for _, lo, hi in BLOCKS]

INT_COLS = 4 * (GWS[0] + GWS[1])              # c0..c3 of blocks 0,1
FP8_COLS = 4 * (GWS[0] + GWS[1]) + 8 * (GWS[2] + GWS[3])

F32 = mybir.dt.float32
F16 = mybir.dt.float16
I8 = mybir.dt.int8
FP8 = mybir.dt.float8e4
NP_F16 = np.float16
NP_FP8 = ml_dtypes.float8_e4m3fn

_cached = None
last_results = None


def _dct2(N):
    n = np.arange(N, dtype=np.float64)
    k = np.arange(N, dtype=np.float64)
    return 2.0 * np.cos(np.pi * (2.0 * n[None, :] + 1.0) * k[:, None] / (2.0 * N))


# host-side piece list: (kind, block, name, c-slices)
# int pieces come from xi (c-slices of 0..3), fp8 pieces from xf (0..3 = c4..c7)
PIECES = [
    ("f", 0, "f8a", (0, 1)),
    ("i", 0, "c01", (0, 1)),
    ("i", 0, "c23", (2, 3)),
    ("f", 0, "f8b", (2, 3)),
    ("f", 1, "f8a", (0, 1)),
    ("i", 1, "c01", (0, 1)),
    ("i", 1, "c23", (2, 3)),
    ("f", 1, "f8b", (2, 3)),
    ("g", 2, "gA", (0, 1, 2, 3)),
    ("g", 2, "gB", (4, 5, 6, 7)),
    ("g", 3, "gA", (0, 1, 2, 3)),
    ("g", 3, "gB", (4, 5, 6, 7)),
]

# per-block column base in the block-major w layout
W_BASE = [0]
for _s, _lo, _hi in BLOCKS:
    W_BASE.append(W_BASE[-1] + (_hi - _lo) * NF)
W01_COLS = W_BASE[2]              # blocks 0+1
W23_COLS = W_BASE[4] - W_BASE[2]  # blocks 2+3


def _kernel_body(tc, xi, xf, w, bias, out):
    nc = tc.nc
    with ExitStack() as ctx:
        const_pool = ctx.enter_context(tc.tile_pool(name="const", bufs=1))
        xpool = ctx.enter_context(tc.tile_pool(name="xp", bufs=1))
        upool = ctx.enter_context(tc.tile_pool(name="up", bufs=1))
        pft_pool = ctx.enter_context(tc.tile_pool(name="pft", bufs=1, space="PSUM"))

        # consts ride the scalar (ACT) HWDGE ring - off the x stream ring.
        # w comes in 2 block-major slices: one 512 KB lump gets round-robin
        # interleaved into the x stream at a different point per SDMA engine
        # (~2 us of piece-completion skew), while 4+ small DMAs overflow the
        # ~6-deep queue and stall the descriptor generator.
        w01_sb = const_pool.tile([P, W01_COLS], F16, tag="w01", name="w01")
        nc.scalar.dma_start(out=w01_sb, in_=w[:, 0:W01_COLS])
        w23_sb = const_pool.tile([P, W23_COLS], F16, tag="w23", name="w23")
        nc.scalar.dma_start(out=w23_sb, in_=w[:, W01_COLS:W01_COLS + W23_COLS])
        w_sb = []
        for bi in range(4):
            src = w01_sb if bi < 2 else w23_sb
            base = W_BASE[bi] - (0 if bi < 2 else W01_COLS)
            w_sb.append((src, base))
        bias_sb = const_pool.tile([OUT_F, 1], F32)
        nc.scalar.dma_start(out=bias_sb, in_=bias)
        alpha_sb = const_pool.tile([OUT_F, 1], F32)
        nc.gpsimd.memset(alpha_sb, SLOPE)
        # prime the ACT Prelu table so ACT_TABLE_LOAD (~1.3us) runs early
        prime_sb = const_pool.tile([OUT_F, 1], F32)
        nc.scalar.activation(
            prime_sb,
            alpha_sb,
            mybir.ActivationFunctionType.Prelu,
            bias=alpha_sb,
            alpha=alpha_sb,
        )

        out_sb = const_pool.tile([OUT_F, BS], F32)
        psum_feat = [
            pft_pool.tile([NF, BS], F32, tag=f"feat{s}", name=f"psum_feat{s}")
            for s in range(NSW)
        ]

        # pre-warm the PE: the HAM clock gate needs ~3.4us of sustained
        # activity to ramp 1.2 -> 2.4 GHz; burn dummy matmuls on a memset
        # tile so the real matmuls start warm (56 vs 107 ns each).
        warm_sb = const_pool.tile([P, NF], F16, tag="warm", name="warm")
        nc.gpsimd.memset(warm_sb, 0.0)
        psum_warm = pft_pool.tile([NF, NF], F32, tag="warm", name="psum_warm")
        NWARM = 32
        for i in range(NWARM):
            nc.tensor.matmul(
                psum_warm, lhsT=warm_sb, rhs=warm_sb,
                start=(i == 0), stop=(i == NWARM - 1),
            )
        # prime the DVE as well (first DVE op pays ~0.3us of ucode warmup)
        warm_v = const_pool.tile([P, NF], F16, tag="warmv", name="warmv")
        nc.vector.tensor_add(out=warm_v, in0=warm_sb, in1=warm_sb)

        # stream pieces, strict FIFO on the sync ring
        tiles = {}
        ioff = foff = 0
        for kind, bi, nm, cs in PIECES:
            gw = GWS[bi]
            ncols = len(cs) * gw
            if kind == "i":
                t = xpool.tile([P, ncols], I8, tag=f"x{bi}{nm}", name=f"x{bi}{nm}")
                nc.sync.dma_start(out=t, in_=xi[:, ioff:ioff + ncols])
                ioff += ncols
            else:
                t = xpool.tile([P, ncols], FP8, tag=f"x{bi}{nm}", name=f"x{bi}{nm}")
                nc.sync.dma_start(out=t, in_=xf[:, foff:foff + ncols])
                foff += ncols
            tiles[(bi, nm)] = t

        # --- adds (issue order = measured readiness; queues are in-order) --
        def pair_add(eng, nm, bi, src, off):
            gw = GWS[bi]
            t = upool.tile([P, gw], F16, tag=f"{nm}_{bi}", name=f"{nm}_{bi}")
            eng.tensor_add(
                out=t, in0=src[:, off * gw:(off + 1) * gw],
                in1=src[:, (off + 1) * gw:(off + 2) * gw],
            )
            return t

        # gpsimd: u23 for the big s1 block only (Q7 is ~2x slower than DVE)
        u23_0 = pair_add(nc.gpsimd, "u23", 0, tiles[(0, "c23")], 0)
        # DVE (issue order = measured readiness)
        u01_0 = pair_add(nc.vector, "u01", 0, tiles[(0, "c01")], 0)
        v0_0 = upool.tile([P, GWS[0]], F16, tag="v0_0", name="v0_0")
        nc.vector.tensor_add(out=v0_0, in0=u01_0, in1=u23_0)
        u01_1 = pair_add(nc.vector, "u01", 1, tiles[(1, "c01")], 0)
        u23_1 = pair_add(nc.vector, "u23", 1, tiles[(1, "c23")], 0)

        # --- matmuls ------------------------------------------------------
        n_mm = {0: 0, 1: 0}
        for bi, (s, lo, hi) in enumerate(BLOCKS):
            nch = hi - lo
            if bi == 0:
                n_mm[s] += 5 * nch
            elif bi == 1:
                n_mm[s] += 6 * nch
            else:
                n_mm[s] += 8 * nch
        mm_seen = {0: 0, 1: 0}

        def mm(bi, chin, rhs):
            s = BLOCKS[bi][0]
            wt, base = w_sb[bi]
            mm_seen[s] += 1
            nc.tensor.matmul(
                psum_feat[s],
                lhsT=wt[:, base + chin * NF:base + (chin + 1) * NF],
                rhs=rhs,
                start=(mm_seen[s] == 1),
                stop=(mm_seen[s] == n_mm[s]),
            )

        def fp8_group(bi, nm, njs):
            _, lo, hi = BLOCKS[bi]
            gw = GWS[bi]
            f8 = tiles[(bi, nm)]
            for j in range(njs):
                for chin in range(hi - lo):
                    mm(bi, chin, f8[:, j * gw + chin * P:j * gw + (chin + 1) * P])

        def red_group(bi, src):
            _, lo, hi = BLOCKS[bi]
            for chin in range(hi - lo):
                mm(bi, chin, src[:, chin * P:(chin + 1) * P])

        # PE queue in expected-readiness order
        fp8_group(0, "f8a", 2)
        fp8_group(0, "f8b", 2)
        fp8_group(1, "f8a", 2)
        red_group(0, v0_0)
        red_group(1, u01_1)
        fp8_group(1, "f8b", 2)
        red_group(1, u23_1)
        fp8_group(2, "gA", 4)
        fp8_group(2, "gB", 4)              # s0 stop
        fp8_group(3, "gA", 4)
        fp8_group(3, "gB", 4)              # s1 stop - zero DVE in the tail

        assert mm_seen[0] == n_mm[0] and mm_seen[1] == n_mm[1]

        # --- epilogue: Prelus back-to-back on ACT, ONE out DMA on sync ----
        for s in range(NSW):
            nc.scalar.activation(
                out_sb[s * NF:(s + 1) * NF, :],
                psum_feat[s],
                mybir.ActivationFunctionType.Prelu,
                bias=bias_sb[s * NF:(s + 1) * NF, :],
                alpha=alpha_sb[s * NF:(s + 1) * NF, :],
            )
        nc.sync.dma_start(out=out, in_=out_sb)


def _build():
    global _cached
    if _cached is not None:
        return _cached
    nc = bacc.Bacc(
        "TRN2",
        target_bir_lowering=False,
        debug=False,
        enable_asserts=False,
        num_devices=NCORES,
    )
    xi_ap = nc.dram_tensor("xi", [P, INT_COLS], I8, kind="ExternalInput").ap()
    xf_ap = nc.dram_tensor("xf", [P, FP8_COLS], FP8, kind="ExternalInput").ap()
    w_ap = nc.dram_tensor("w", [P, NSW * NCHUNK * NF], F16, kind="ExternalInput").ap()
    b_ap = nc.dram_tensor("bias", [OUT_F, 1], F32, kind="ExternalInput").ap()
    out_ap = nc.dram_tensor("out", [OUT_F, BS], F32, kind="ExternalOutput").ap()
    with tile.TileContext(nc, trace_sim=False) as tc:
        _kernel_body(tc, xi_ap, xf_ap, w_ap, b_ap, out_ap)
    nc.compile()
    _cached = nc
    return nc


def kernel(x, W, b):
    global last_results
    assert x.shape == (B, 1, NSW * NBINS * NDCT, HW, HW), x.shape
    nc = _build()

    # Host-side folding of the DCT matrices into the conv weights (tiny).
    Ct = _dct2(NDCT)                       # [f, t]
    Cs = _dct2(HW)                         # [p, i]
    Weff = np.einsum(
        "ft,pi,qj,sofpq->sotij", Ct, Cs, Cs, W.astype(np.float64), optimize=True
    ) / float(NBINS)
    Weff_k = Weff.reshape(NSW, NF, K) * QSCALE   # fold dequant scale
    # block-major lhsT layout: w[p, W_BASE[bi] + chin*NF + o]
    #   = Weff_k[s, o, (lo+chin)*128 + p]
    wc = Weff_k.reshape(NSW, NF, NCHUNK, P)
    blocks_w = []
    for s, lo, hi in BLOCKS:
        blocks_w.append(wc[s, :, lo:hi, :].transpose(2, 1, 0).reshape(P, (hi - lo) * NF))
    w_dev = np.ascontiguousarray(np.concatenate(blocks_w, axis=1)).astype(NP_F16)
    bias_dev = np.ascontiguousarray(b.reshape(OUT_F, 1)).astype(np.float32)

    # Error-feedback quantization along c (fp8 slices first so their larger
    # residuals are absorbed by the later int8 slices).
    xs = x.reshape(B, NSW, NBINS, K).astype(np.float32) / np.float32(QSCALE)
    qi = np.zeros((B, NSW, 4, K), dtype=np.int8)
    qf = np.zeros((B, NSW, 4, K), dtype=NP_FP8)
    e = np.zeros((B, NSW, K), dtype=np.float32)
    for j, c in enumerate((4, 5, 6, 7)):
        v = xs[:, :, c] + e
        qc = v.astype(NP_FP8)
        e = v - qc.astype(np.float32)
        qf[:, :, j] = qc
    for c in range(4):
        v = xs[:, :, c] + e
        qc = np.clip(np.round(v), -127, 127)
        e = v - qc
        qi[:, :, c] = qc.astype(np.int8)

    # all-fp8 chain for the two tail blocks (no int pieces -> no DVE work
    # after stream end; final residual is fp8 on 8/32 of the chunks, total
    # rel err ~5.4e-3, still 3.7x under the gate)
    qg = np.zeros((B, NSW, 8, K), dtype=NP_FP8)
    e = np.zeros((B, NSW, K), dtype=np.float32)
    for c in range(8):
        v = xs[:, :, c] + e
        qc = v.astype(NP_FP8)
        e = v - qc.astype(np.float32)
        qg[:, :, c] = qc

    qi = qi.reshape(B, NSW, 4, NCHUNK, P)
    qf = qf.reshape(B, NSW, 4, NCHUNK, P)
    qg = qg.reshape(B, NSW, 8, NCHUNK, P)

    in_maps = []
    for i in range(NCORES):
        sl = slice(i * BS, (i + 1) * BS)
        icols, fcols = [], []
        for kind, bi, nm, cs in PIECES:
            s, lo, hi = BLOCKS[bi]
            src = {"i": qi, "f": qf, "g": qg}[kind]
            t = src[sl, s][:, list(cs), lo:hi, :]        # [b, nc, ch, kin]
            t = t.transpose(3, 1, 2, 0).reshape(P, len(cs) * (hi - lo) * BS)
            (icols if kind == "i" else fcols).append(t)
        xi_dev = np.ascontiguousarray(np.concatenate(icols, axis=1))
        xf_dev = np.ascontiguousarray(np.concatenate(fcols, axis=1))
        in_maps.append({"xi": xi_dev, "xf": xf_dev, "w": w_dev, "bias": bias_dev})
    res = run_bass_kernel_spmd(nc, in_maps, core_ids=list(range(NCORES)))
    last_results = res
    # device emits [s*64+o, b] per core; transpose back to [b, s*64+o]
    return np.concatenate([r["out"].T for r in res.results], axis=0)


# revision 35
# speedup vs baseline: 1.2048x; 1.0239x over previous
"""Trainium2 Bass kernel for nn_DCTFeatureModel.

Math: the reference pipeline (3D DCT-II over [time-in-bin, H, W], mean over
DCT bins, full-receptive-field Conv3d, bias, LeakyReLU) is linear up to the
LeakyReLU, so everything folds into a single small matmul:

    feat[b,s,o] = LeakyReLU( sum_{c,t,i,j} x[b,s,c,t,i,j] * Weff[s,o,t,i,j]
                             + bias[s,o] )
    Weff[s,o,t,i,j] = (1/8) * sum_{f,p,q} Ct[f,t] Cs[p,i] Cs[q,j] W[s,o,f,p,q]

Weff is tiny and computed on host.

Quantization (rel-err budget 2e-2; this lands ~3.3e-3): x ships at 1
byte/element in "count units" x/s (s = 4/127): DCT bins c0..c3 as int8,
c4..c7 as fp8e4. Error-feedback quantization along the c chain (fp8 slices
first, int8 after) makes the device's c-sum accurate to the final int8
residual (~0.3%). The dequant scale s is folded into the fp16 weights.

Device schedule (final, ~33 us vs 46.7 us bf16 baseline; chip-level
HBM-bound - 8 cores pull ~2.3 TB/s aggregate):
- 12 x-pieces on the sync HWDGE ring, each big block led by half its fp8
  so the PE starts early; block sizes s1:13+3, s0:11+5 so late-arriving
  pieces carry little PE work. Piece-completion sems lag the data by the
  slowest SDMA engine, so every consumer queue is issued in
  measured-readiness order (all queues are in-order).
- w rides the scalar HWDGE ring in 2 block-major slices: one 512 KB lump
  gets round-robin interleaved into the x stream at a different point per
  SDMA engine (~2 us sem skew); 4+ small DMAs overflow the ~6-deep queue.
- PE contracts the fp8 slices RAW (fp16 lhsT x fp8 rhs matmuls accumulate
  the c-sum in PSUM; HW-verified exact): 179 matmuls, 56-107 ns each,
  LDWEIGHTS fully hidden; 32 dummy matmuls at start ramp the HAM clock
  gate 1.2 -> 2.4 GHz before real work arrives.
- DVE: u01=c0+c1 everywhere (int8->fp16 ~1.3 ns/col), u23 for the three
  later blocks, v0=u01+u23 for block 0 (fp16 2x). GpSimd: u23 of the big
  s1 block (~2.5 ns/col). Unreduced u01/u23 go straight to the PE.
- ACT Prelu table primed at start (else its 1.3 us load lands in the
  tail); both Prelus back-to-back at the end; one merged out DMA.

Sharding: pure data-parallel over batch, 1024/8 = 128 rows per core.
"""

from contextlib import ExitStack

import ml_dtypes
import numpy as np

import concourse.bacc as bacc
import concourse.tile as tile
from concourse import mybir
from concourse.bass_utils import run_bass_kernel_spmd

# Problem shapes (hardcoded per contract)
B = 1024
NCORES = 8
BS = B // NCORES          # 128 batch rows per core
NSW = 2                   # subwindows
NBINS = 8                 # DCT bins (mean-reduced)
NDCT = 32                 # time points per bin
HW = 8
NF = 64                   # conv output filters per subwindow
K = NDCT * HW * HW        # 2048 contraction elements per (s, c)
P = 128                   # partitions
NCHUNK = K // P           # 16 k-chunks of 128 per subwindow
OUT_F = NSW * NF          # 128 output features
SLOPE = 0.02
QSCALE = 4.0 / 127.0      # int8 quant scale (4-sigma clip)

# blocks in STREAM order: (s, chunk_lo, chunk_hi)
BLOCKS = [(1, 0, 13), (0, 0, 11), (0, 11, 16), (1, 13, 16)]
GWS = [(hi - lo) * P for _, lo, hi in BLOCKS]

INT_COLS = sum(4 * gw for gw in GWS)   # c0..c3
FP8_COLS = sum(4 * gw for gw in GWS)   # c4..c7

F32 = mybir.dt.float32
F16 = mybir.dt.float16
I8 = mybir.dt.int8
FP8 = mybir.dt.float8e4
NP_F16 = np.float16
NP_FP8 = ml_dtypes.float8_e4m3fn

_cached = None
last_results = None


def _dct2(N):
    n = np.arange(N, dtype=np.float64)
    k = np.arange(N, dtype=np.float64)
    return 2.0 * np.cos(np.pi * (2.0 * n[None, :] + 1.0) * k[:, None] / (2.0 * N))


# host-side piece list: (kind, block, name, c-slices)
# int pieces come from xi (c-slices of 0..3), fp8 pieces from xf (0..3 = c4..c7)
PIECES = [
    ("f", 0, "f8a", (0, 1)),
    ("i", 0, "c01", (0, 1)),
    ("i", 0, "c23", (2, 3)),
    ("f", 0, "f8b", (2, 3)),
    ("f", 1, "f8a", (0, 1)),
    ("i", 1, "c01", (0, 1)),
    ("i", 1, "c23", (2, 3)),
    ("f", 1, "f8b", (2, 3)),
    ("i", 2, "int", (0, 1, 2, 3)),
    ("f", 2, "f8", (0, 1, 2, 3)),
    ("i", 3, "int", (0, 1, 2, 3)),
    ("f", 3, "f8", (0, 1, 2, 3)),
]

# per-block column base in the block-major w layout
W_BASE = [0]
for _s, _lo, _hi in BLOCKS:
    W_BASE.append(W_BASE[-1] + (_hi - _lo) * NF)
W01_COLS = W_BASE[2]              # blocks 0+1
W23_COLS = W_BASE[4] - W_BASE[2]  # blocks 2+3


def _kernel_body(tc, xi, xf, w, bias, out):
    nc = tc.nc
    with ExitStack() as ctx:
        const_pool = ctx.enter_context(tc.tile_pool(name="const", bufs=1))
        xpool = ctx.enter_context(tc.tile_pool(name="xp", bufs=1))
        upool = ctx.enter_context(tc.tile_pool(name="up", bufs=1))
        pft_pool = ctx.enter_context(tc.tile_pool(name="pft", bufs=1, space="PSUM"))

        # consts ride the scalar (ACT) HWDGE ring - off the x stream ring.
        # w comes in 2 block-major slices: one 512 KB lump gets round-robin
        # interleaved into the x stream at a different point per SDMA engine
        # (~2 us of piece-completion skew), while 4+ small DMAs overflow the
        # ~6-deep queue and stall the descriptor generator.
        w01_sb = const_pool.tile([P, W01_COLS], F16, tag="w01", name="w01")
        nc.scalar.dma_start(out=w01_sb, in_=w[:, 0:W01_COLS])
        w23_sb = const_pool.tile([P, W23_COLS], F16, tag="w23", name="w23")
        nc.scalar.dma_start(out=w23_sb, in_=w[:, W01_COLS:W01_COLS + W23_COLS])
        w_sb = []
        for bi in range(4):
            src = w01_sb if bi < 2 else w23_sb
            base = W_BASE[bi] - (0 if bi < 2 else W01_COLS)
            w_sb.append((src, base))
        bias_sb = const_pool.tile([OUT_F, 1], F32)
        nc.scalar.dma_start(out=bias_sb, in_=bias)
        alpha_sb = const_pool.tile([OUT_F, 1], F32)
        nc.gpsimd.memset(alpha_sb, SLOPE)
        # prime the ACT Prelu table so ACT_TABLE_LOAD (~1.3us) runs early
        prime_sb = const_pool.tile([OUT_F, 1], F32)
        nc.scalar.activation(
            prime_sb,
            alpha_sb,
            mybir.ActivationFunctionType.Prelu,
            bias=alpha_sb,
            alpha=alpha_sb,
        )

        out_sb = const_pool.tile([OUT_F, BS], F32)
        psum_feat = [
            pft_pool.tile([NF, BS], F32, tag=f"feat{s}", name=f"psum_feat{s}")
            for s in range(NSW)
        ]

        # pre-warm the PE: the HAM clock gate needs ~3.4us of sustained
        # activity to ramp 1.2 -> 2.4 GHz; burn dummy matmuls on a memset
        # tile so the real matmuls start warm (56 vs 107 ns each).
        warm_sb = const_pool.tile([P, NF], F16, tag="warm", name="warm")
        nc.gpsimd.memset(warm_sb, 0.0)
        psum_warm = pft_pool.tile([NF, NF], F32, tag="warm", name="psum_warm")
        NWARM = 32
        for i in range(NWARM):
            nc.tensor.matmul(
                psum_warm, lhsT=warm_sb, rhs=warm_sb,
                start=(i == 0), stop=(i == NWARM - 1),
            )
        # prime the DVE as well (first DVE op pays ~0.3us of ucode warmup)
        warm_v = const_pool.tile([P, NF], F16, tag="warmv", name="warmv")
        nc.vector.tensor_add(out=warm_v, in0=warm_sb, in1=warm_sb)

        # stream pieces, strict FIFO on the sync ring
        tiles = {}
        ioff = foff = 0
        for kind, bi, nm, cs in PIECES:
            gw = GWS[bi]
            ncols = len(cs) * gw
            if kind == "i":
                t = xpool.tile([P, ncols], I8, tag=f"x{bi}{nm}", name=f"x{bi}{nm}")
                nc.sync.dma_start(out=t, in_=xi[:, ioff:ioff + ncols])
                ioff += ncols
            else:
                t = xpool.tile([P, ncols], FP8, tag=f"x{bi}{nm}", name=f"x{bi}{nm}")
                nc.sync.dma_start(out=t, in_=xf[:, foff:foff + ncols])
                foff += ncols
            tiles[(bi, nm)] = t

        # --- adds (issue order = measured readiness; queues are in-order) --
        def pair_add(eng, nm, bi, src, off):
            gw = GWS[bi]
            t = upool.tile([P, gw], F16, tag=f"{nm}_{bi}", name=f"{nm}_{bi}")
            eng.tensor_add(
                out=t, in0=src[:, off * gw:(off + 1) * gw],
                in1=src[:, (off + 1) * gw:(off + 2) * gw],
            )
            return t

        # gpsimd: u23 for the big s1 block only (Q7 is ~2x slower than DVE)
        u23_0 = pair_add(nc.gpsimd, "u23", 0, tiles[(0, "c23")], 0)
        # DVE (issue order = measured readiness)
        u01_0 = pair_add(nc.vector, "u01", 0, tiles[(0, "c01")], 0)
        v0_0 = upool.tile([P, GWS[0]], F16, tag="v0_0", name="v0_0")
        nc.vector.tensor_add(out=v0_0, in0=u01_0, in1=u23_0)
        u01_1 = pair_add(nc.vector, "u01", 1, tiles[(1, "c01")], 0)
        u23_1 = pair_add(nc.vector, "u23", 1, tiles[(1, "c23")], 0)
        u01_2 = pair_add(nc.vector, "u01", 2, tiles[(2, "int")], 0)
        u23_2 = pair_add(nc.vector, "u23", 2, tiles[(2, "int")], 2)
        u01_3 = pair_add(nc.vector, "u01", 3, tiles[(3, "int")], 0)
        u23_3 = pair_add(nc.vector, "u23", 3, tiles[(3, "int")], 2)

        # --- matmuls ------------------------------------------------------
        n_mm = {0: 0, 1: 0}
        for bi, (s, lo, hi) in enumerate(BLOCKS):
            nch = hi - lo
            n_mm[s] += 4 * nch + (nch if bi == 0 else 2 * nch)
        mm_seen = {0: 0, 1: 0}

        def mm(bi, chin, rhs):
            s = BLOCKS[bi][0]
            wt, base = w_sb[bi]
            mm_seen[s] += 1
            nc.tensor.matmul(
                psum_feat[s],
                lhsT=wt[:, base + chin * NF:base + (chin + 1) * NF],
                rhs=rhs,
                start=(mm_seen[s] == 1),
                stop=(mm_seen[s] == n_mm[s]),
            )

        def fp8_group(bi, nm, njs):
            _, lo, hi = BLOCKS[bi]
            gw = GWS[bi]
            f8 = tiles[(bi, nm)]
            for j in range(njs):
                for chin in range(hi - lo):
                    mm(bi, chin, f8[:, j * gw + chin * P:j * gw + (chin + 1) * P])

        def red_group(bi, src):
            _, lo, hi = BLOCKS[bi]
            for chin in range(hi - lo):
                mm(bi, chin, src[:, chin * P:(chin + 1) * P])

        # PE queue in expected-readiness order
        fp8_group(0, "f8a", 2)
        fp8_group(0, "f8b", 2)
        fp8_group(1, "f8a", 2)
        red_group(0, v0_0)
        red_group(1, u01_1)
        fp8_group(1, "f8b", 2)
        red_group(1, u23_1)
        fp8_group(2, "f8", 4)
        fp8_group(3, "f8", 4)              # DMA-gated only - before the
        red_group(2, u01_2)                # DVE-gated red groups (measured:
        red_group(2, u23_2)                # s0 stop      the DVE chain ends
        red_group(3, u01_3)                # ~2.5us after the last piece)
        red_group(3, u23_3)                # s1 stop

        assert mm_seen[0] == n_mm[0] and mm_seen[1] == n_mm[1]

        # --- epilogue: Prelus back-to-back on ACT, ONE out DMA on sync ----
        for s in range(NSW):
            nc.scalar.activation(
                out_sb[s * NF:(s + 1) * NF, :],
                psum_feat[s],
                mybir.ActivationFunctionType.Prelu,
                bias=bias_sb[s * NF:(s + 1) * NF, :],
                alpha=alpha_sb[s * NF:(s + 1) * NF, :],
            )
        nc.sync.dma_start(out=out, in_=out_sb)


def _build():
    global _cached
    if _cached is not None:
        return _cached
    nc = bacc.Bacc(
        "TRN2",
        target_bir_lowering=False,
        debug=False,
        enable_asserts=False,
        num_devices=NCORES,
    )
    xi_ap = nc.dram_tensor("xi", [P, INT_COLS], I8, kind="ExternalInput").ap()
    xf_ap = nc.dram_tensor("xf", [P, FP8_COLS], FP8, kind="ExternalInput").ap()
    w_ap = nc.dram_tensor("w", [P, NSW * NCHUNK * NF], F16, kind="ExternalInput").ap()
    b_ap = nc.dram_tensor("bias", [OUT_F, 1], F32, kind="ExternalInput").ap()
    out_ap = nc.dram_tensor("out", [OUT_F, BS], F32, kind="ExternalOutput").ap()
    with tile.TileContext(nc, trace_sim=False) as tc:
        _kernel_body(tc, xi_ap, xf_ap, w_ap, b_ap, out_ap)
    nc.compile()
    _cached = nc
    return nc


def kernel(x, W, b):
    global last_results
    assert x.shape == (B, 1, NSW * NBINS * NDCT, HW, HW), x.shape
    nc = _build()

    # Host-side folding of the DCT matrices into the conv weights (tiny).
    Ct = _dct2(NDCT)                       # [f, t]
    Cs = _dct2(HW)                         # [p, i]
    Weff = np.einsum(
        "ft,pi,qj,sofpq->sotij", Ct, Cs, Cs, W.astype(np.float64), optimize=True
    ) / float(NBINS)
    Weff_k = Weff.reshape(NSW, NF, K) * QSCALE   # fold dequant scale
    # block-major lhsT layout: w[p, W_BASE[bi] + chin*NF + o]
    #   = Weff_k[s, o, (lo+chin)*128 + p]
    wc = Weff_k.reshape(NSW, NF, NCHUNK, P)
    blocks_w = []
    for s, lo, hi in BLOCKS:
        blocks_w.append(wc[s, :, lo:hi, :].transpose(2, 1, 0).reshape(P, (hi - lo) * NF))
    w_dev = np.ascontiguousarray(np.concatenate(blocks_w, axis=1)).astype(NP_F16)
    bias_dev = np.ascontiguousarray(b.reshape(OUT_F, 1)).astype(np.float32)

    # Error-feedback quantization along c (fp8 slices first so their larger
    # residuals are absorbed by the later int8 slices).
    xs = x.reshape(B, NSW, NBINS, K).astype(np.float32) / np.float32(QSCALE)
    qi = np.zeros((B, NSW, 4, K), dtype=np.int8)
    qf = np.zeros((B, NSW, 4, K), dtype=NP_FP8)
    e = np.zeros((B, NSW, K), dtype=np.float32)
    for j, c in enumerate((4, 5, 6, 7)):
        v = xs[:, :, c] + e
        qc = v.astype(NP_FP8)
        e = v - qc.astype(np.float32)
        qf[:, :, j] = qc
    for c in range(4):
        v = xs[:, :, c] + e
        qc = np.clip(np.round(v), -127, 127)
        e = v - qc
        qi[:, :, c] = qc.astype(np.int8)

    qi = qi.reshape(B, NSW, 4, NCHUNK, P)
    qf = qf.reshape(B, NSW, 4, NCHUNK, P)

    in_maps = []
    for i in range(NCORES):
        sl = slice(i * BS, (i + 1) * BS)
        icols, fcols = [], []
        for kind, bi, nm, cs in PIECES:
            s, lo, hi = BLOCKS[bi]
            src = qi if kind == "i" else qf
            t = src[sl, s][:, list(cs), lo:hi, :]        # [b, nc, ch, kin]
            t = t.transpose(3, 1, 2, 0).reshape(P, len(cs) * (hi - lo) * BS)
            (icols if kind == "i" else fcols).append(t)
        xi_dev = np.ascontiguousarray(np.concatenate(icols, axis=1))
        xf_dev = np.ascontiguousarray(np.concatenate(fcols, axis=1))
        in_maps.append({"xi": xi_dev, "xf": xf_dev, "w": w_dev, "bias": bias_dev})
    res = run_bass_kernel_spmd(nc, in_maps, core_ids=list(range(NCORES)))
    last_results = res
    # device emits [s*64+o, b] per core; transpose back to [b, s*64+o]
    return np.concatenate([r["out"].T for r in res.results], axis=0)
